# revision 46
# baseline (speedup 1.0000x reference)
"""Trainium2 Bass kernel for nn_BDHBlock (pre-LN latent block with
softmax-free attention and sigmoid gating).

Sharding: data-parallel over batch B=16 across 8 cores (2 per core).
No collectives; outputs are concatenated on the host.

Per-core math (B_loc=2, N=1024, D=768, H=12, HD=64), all matmuls fp16
with fp32 PSUM accumulation:
  xn   = LayerNorm(x)                      (ln_w/ln_b folded into
                                            enc/gate weights host-side)
  lat  = relu(xn @ enc_w'.T + enc_b')      (feature-major)
  qk   = rope(lat @ qk_w.T + qk_b) / sqrt(sqrt(HD))   (token-major)
  v    = lat @ v_w.T + v_b                 (token-major)
  T_h  = qk_h^T @ v_h     per (b, head-pair), fused 128-wide matmuls
  attn_h = qk_h @ T_h     (== (qk qk^T/8) v by associativity)
  out  = x + sigmoid(xn @ gate_w'.T + gate_b') * (attn @ out_w.T + out_b)

Layout notes:
- All SBUF<->SBUF transposes (xn -> xnT, qk -> qkT) go through the DMA
  XBAR transpose (fp16), not the PE array.
- Weights are loaded with casting gpsimd DMAs (f32 DRAM -> f16 SBUF).
- qk output features are host-permuted per 384-block to [all-even dims,
  all-odd dims] so the rope multiplies are contiguous 2D ops; the
  gpsimd add/sub writes un-permute back to per-head [rot1|rot2] order.
"""

import os
import sys

for _p in ("/opt/trn_rl_repo", "/root/.axon_site/_ro/trn_rl_repo"):
    if os.path.isdir(_p) and _p not in sys.path:
        sys.path.insert(0, _p)

import math
import numpy as np

import concourse.bass as bass
import concourse.mybir as mybir
from concourse import bacc
from concourse import bass_utils
from concourse.bass import ts, ds
from concourse.tile import TileContext

F32 = mybir.dt.float32
F16 = mybir.dt.float16
AF = mybir.ActivationFunctionType

P = 128          # partitions
D = 768
KT = D // P      # 6 d-tiles
B_LOC = 2        # batch elements per core
SEQ = 1024
T = B_LOC * SEQ  # 2048 tokens per core
NT = T // P      # 16 token tiles
TPB = SEQ // P   # 8 token tiles per batch element
TW = 512         # token window (feature-major matmul free dim)
NTW = T // TW    # 4
JW = 384         # feature window (token-major matmul free dim)
NJW = D // JW    # 2
H = 12
HD = 64
EPS = 1e-5
QK_SCALE = 1.0 / math.sqrt(math.sqrt(HD))  # applied twice => 1/sqrt(HD)

W_NAMES = ["enc_w", "qk_w", "v_w", "out_w", "gate_w"]


def _trig_coefs():
    """Power-series coefficients for sin(x)=x*S(x^2), cos(x)=C(x^2) on
    |x|<=6 (rope_emb is N(0,1); the ACT Sin LUT is unusable there)."""
    xs = np.linspace(1e-8, 6.0, 40001)
    u = xs ** 2
    cheb = np.polynomial.chebyshev
    for deg in range(7, 14):
        s = cheb.cheb2poly(cheb.chebfit(u, np.sin(xs) / xs, deg))
        c = cheb.cheb2poly(cheb.chebfit(u, np.cos(xs), deg))
        es = np.abs(np.polyval(s[::-1], u) * xs - np.sin(xs)).max()
        ec = np.abs(np.polyval(c[::-1], u) - np.cos(xs)).max()
        if max(es, ec) < 4e-5:
            break
    return [float(v) for v in s], [float(v) for v in c]


SIN_COEF, COS_COEF = _trig_coefs()


def build_nc():
    nc = bacc.Bacc("TRN2", target_bir_lowering=False, debug=False)

    x_in = nc.dram_tensor("x", [B_LOC, SEQ, D], F32, kind="ExternalInput")
    rope_in = nc.dram_tensor("rope_emb", [SEQ, HD], F32, kind="ExternalInput")
    vecs = {}
    for nm in ["enc_b", "qk_b", "v_b", "out_b", "gate_b"]:
        vecs[nm] = nc.dram_tensor(nm, [D], F32, kind="ExternalInput")
    # weights arrive host-cast to fp16 (and host-transposed): halves DMA
    # bytes and removes on-device casts
    w_in = {nm: nc.dram_tensor(nm, [D, D], F16, kind="ExternalInput")
            for nm in W_NAMES}
    out_t = nc.dram_tensor("out", [B_LOC, SEQ, D], F32, kind="ExternalOutput")

    x_flat = x_in.ap().rearrange("b n d -> (b n) d")
    out_flat = out_t.ap().rearrange("b n d -> (b n) d")

    with TileContext(nc) as tc:
        with (
            tc.tile_pool(name="consts", bufs=1) as cp,
            tc.tile_pool(name="wrot", bufs=3) as wrot,
            tc.tile_pool(name="big", bufs=2) as bigp,
            tc.tile_pool(name="bigh", bufs=3) as bigh,
            tc.tile_pool(name="xin", bufs=2) as xip,
            tc.tile_pool(name="xres", bufs=2) as xrp,
            tc.tile_pool(name="xn16", bufs=2) as xnp,
            tc.tile_pool(name="stats", bufs=2) as stp,
            tc.tile_pool(name="qkp", bufs=2) as qpp,
            tc.tile_pool(name="ropem", bufs=4) as rmp,
            tc.tile_pool(name="t16", bufs=2) as t16p,
            tc.tile_pool(name="g16", bufs=2) as g16p,
            tc.tile_pool(name="ao", bufs=2) as aop,
            tc.tile_pool(name="ps512", bufs=3, space="PSUM") as ps512,
            tc.tile_pool(name="ps384", bufs=3, space="PSUM") as ps384,
            tc.tile_pool(name="psA", bufs=2, space="PSUM") as psA,
        ):
            # ---------------- constants / weight loads ----------------
            x_src = x_in.ap().rearrange(
                "b (w c p) d -> (b w) p c d", p=P, c=4)
            with nc.named_scope("prep"):
                # x window 0 first, split per tile: LN tile 0 is the whole
                # kernel's prologue and must not wait a 4-tile transfer
                xt4 = xip.tile([P, 4, D], F32, tag="xin", name="xw0")
                for c in range(4):
                    nc.sync.dma_start(xt4[:, c], x_src[0][:, c])
                eps_t = cp.tile([P, 1], F32, tag="epsc")
                nc.vector.memset(eps_t[:], EPS)
                # block-diagonal head mask for the fused 2-head M1 drain
                mask = cp.tile([P, P], F16, tag="hmask")
                nc.vector.memset(mask[:], 0.0)
                nc.vector.memset(mask[0:HD, 0:HD], 1.0)
                nc.vector.memset(mask[HD:P, HD:P], 1.0)
                ones1 = cp.tile([1, P], F16, tag="ones1")
                nc.vector.memset(ones1[:], 1.0)

                # enc bias, per-partition layout [128, KT]
                encb = cp.tile([P, KT], F32, tag="encb")
                nc.sync.dma_start(
                    encb[:], vecs["enc_b"].ap().rearrange("(k p) -> p k", p=P))
                rp = cp.tile([P, TPB, HD], F32, tag="ropein")
                nc.sync.dma_start(
                    rp[:], rope_in.ap().rearrange("(t p) d -> p t d", p=P))
                # qk bias as a broadcast tile: folded into the drain add
                bc_qkb = cp.tile([P, D], F16, tag="bc_qkb")
                nc.gpsimd.dma_start(
                    out=bc_qkb[:],
                    in_=vecs["qk_b"].ap()[None, :].to_broadcast((P, D)))

                # weights: casting DMA (f32 DRAM -> f16 SBUF), whole weight
                # in one descriptor batch; host has pre-transposed to W^T.
                # ring of 3 slots: enc->out, qk->gate, v (alloc order)
                wT = {}
                for nm in W_NAMES:
                    wT[nm] = wrot.tile([P, KT, D], F16, tag="wT",
                                       name=f"wT_{nm}")
                w_src = {nm: w_in[nm].ap().rearrange("(k p) j -> p k j", p=P)
                         for nm in W_NAMES}
                # enc_w per k-tile on sync: enc w0's k-loop can start as
                # soon as the k=0 slice lands
                for k in range(KT):
                    nc.sync.dma_start(wT["enc_w"][:, k, :],
                                      w_src["enc_w"][:, k, :])
                for nm in ["qk_w", "v_w"]:
                    nc.gpsimd.dma_start(wT[nm][:], w_src[nm])

                # bias rows for the K=1 ones-matmul trick
                brow = {}
                for nm in ["v_b", "gate_b", "out_b"]:
                    brow[nm] = cp.tile([1, D], F16, tag=f"brow_{nm}",
                                       name=f"brow_{nm}")
                    nc.gpsimd.dma_start(brow[nm][:], vecs[nm].ap()[None, :])

            # ------------- rope tables (gpsimd, off the LN path) -------
            # tabs: 4 tables [P, TPB, 6*32] fp16, head-replicated so the
            # rope multiplies are plain 2D [128, 192] ops.
            #   cpE=cos(even)*s  spE=sin(even)*s  spO=sin(odd)*s  cpO=cos(odd)*s
            NH = JW // HD  # 6 heads per block
            tabs = {}
            for tn in ["cpE", "spE", "spO", "cpO"]:
                tabs[tn] = cp.tile([P, TPB, NH * (HD // 2)], F16, tag=tn,
                                   name=f"tab_{tn}")
            # Trig tables: f32 Horner on DVE (gpsimd is ~8x slower at
            # tensor_scalar; fp16 Horner loses ~0.5 abs err to cancellation
            # at large u).  Emitted as closures interleaved into the back
            # half of the LN loop so the serial chain hides in DVE slack.
            u = cp.tile([P, TPB, HD], F32, tag="ropeu")
            sin_a = cp.tile([P, TPB, HD], F32, tag="ropesin")
            cos_a = cp.tile([P, TPB, HD], F32, tag="ropecos")
            trig_steps = []

            def _trig_build():
                yield lambda: nc.vector.tensor_mul(u[:], rp[:], rp[:])

                def horner_steps(coef, out):
                    yield lambda: nc.vector.tensor_scalar(
                        out[:], u[:], coef[-1], coef[-2],
                        op0=mybir.AluOpType.mult, op1=mybir.AluOpType.add)
                    for cf in coef[-3::-1]:
                        yield lambda cf=cf: (
                            nc.vector.tensor_mul(out[:], out[:], u[:]),
                            nc.vector.tensor_scalar_add(out[:], out[:], cf))

                yield from horner_steps(SIN_COEF, sin_a)
                yield lambda: nc.vector.tensor_mul(sin_a[:], sin_a[:], rp[:])
                yield from horner_steps(COS_COEF, cos_a)
                srcs = {"cpE": (cos_a, 0), "spE": (sin_a, 0),
                        "spO": (sin_a, 1), "cpO": (cos_a, 1)}
                for tn, (src, par) in srcs.items():
                    def mk(tn=tn, src=src, par=par):
                        dstv = tabs[tn][:].rearrange(
                            "p t (h e) -> p t h e", e=HD // 2)
                        srcv = src[:, :, None, par::2].to_broadcast(
                            (P, TPB, NH, HD // 2))
                        nc.vector.tensor_scalar_mul(dstv, srcv, QK_SCALE)
                    yield mk
            trig_steps = list(_trig_build())

            # xn^T: feature-major [128, KT, T]; lives until the gate GEMMs.
            xnT = cp.tile([P, KT, T], F16, tag="xnT")

            # ---------------- LayerNorm + encoder (interleaved) --------
            # x loads batched 4 tiles per DMA; the full [128,768] -> xnT
            # transpose is one 48-tile XBAR DMA (HWDGE overhead ~630ns/DMA
            # dwarfs the 14ns/tile transfer, so batch aggressively).
            latT = bigp.tile([P, KT, T], F16, tag="big", name="latT")
            xt4s = {0: xt4}
            for i in range(NT):
                with nc.named_scope("ln"):
                    if i % 4 == 0 and i // 4 + 1 < 4:
                        nxt = xip.tile([P, 4, D], F32, tag="xin",
                                       name=f"xw{i // 4 + 1}")
                        nc.sync.dma_start(nxt[:], x_src[i // 4 + 1])
                        xt4s[i // 4 + 1] = nxt
                    xt = xt4s[i // 4][:, i % 4]
                    xg = xt.rearrange("p (s c) -> p s c", c=384)
                    stats = stp.tile([P, 2, 6], F32, tag="bnstats")
                    for s in range(2):
                        nc.vector.bn_stats(stats[:, s, :], xg[:, s, :])
                    mv = stp.tile([P, 2], F32, tag="bnmv")
                    nc.vector.bn_aggr(mv[:], stats[:])
                    rs = stp.tile([P, 1], F32, tag="rstd")
                    nc.scalar.activation(rs[:], mv[:, 1:2], AF.Sqrt,
                                         bias=eps_t[:])
                    nc.vector.reciprocal(rs[:], rs[:])
                    nb = stp.tile([P, 1], F32, tag="negmurs")
                    nc.vector.tensor_scalar(
                        nb[:], mv[:, 0:1], rs[:], -1.0,
                        op0=mybir.AluOpType.mult, op1=mybir.AluOpType.mult)
                    xn16 = xnp.tile([P, D], F16, tag="xn16")
                    nc.scalar.activation(xn16[:], xt, AF.Identity,
                                         bias=nb[:], scale=rs[:])
                    # all transposes on one queue (scalar): the XBAR is a
                    # single shared resource; concurrent transposes from
                    # two queues interleave tiles and corrupt output.
                    # scalar also keeps them off the x-load (sync) queue.
                    nc.scalar.dma_start(xnT[:, :, ts(i, P)], xn16[:],
                                        transpose=True)
                if i >= 8:  # trig chain rides the back half of LN's DVE
                    with nc.named_scope("trig"):
                        for fn in trig_steps[(i - 8) * 3:(i - 7) * 3]:
                            fn()
                if i % 4 == 3:
                    tw = i // 4
                    with nc.named_scope("enc"):
                        for j in range(KT):
                            ps = ps512.tile([P, TW], F32, tag="ps512")
                            for k in range(KT):
                                nc.tensor.matmul(
                                    ps[:], wT["enc_w"][:, k, ts(j, P)],
                                    xnT[:, k, ts(tw, TW)],
                                    start=(k == 0), stop=(k == KT - 1))
                            nc.scalar.activation(latT[:, j, ts(tw, TW)],
                                                 ps[:], AF.Relu,
                                                 bias=encb[:, j:j + 1])

            with nc.named_scope("trig"):
                for fn in trig_steps[(NT - 8) * 3:]:
                    fn()
            # hoist the sigmoid ACT-table swap off the out-phase critical
            # path: last Sqrt user was LN tile 15, first real Sigmoid is in
            # the out phase; swap now while ACT has slack.
            warm_sg = cp.tile([P, 1], F32, tag="warmsg")
            nc.scalar.activation(warm_sg[:], eps_t[:], AF.Sigmoid)

            # late weight loads (slots freed by enc/qk phases)
            nc.gpsimd.dma_start(wT["out_w"][:], w_src["out_w"])
            nc.gpsimd.dma_start(wT["gate_w"][:], w_src["gate_w"])

            # ------------ qk (rope) + v + attention, per batch ---------
            # Per-batch [P, TPB, D]-sized buffers (half of T) ring-reuse
            # between the two batch elements to fit SBUF.
            # M1: fused head-pair [128,128] = qk_pair^T @ v_pair; diag
            # 64-blocks are T_hA, T_hB; off-diag garbage masked at drain.
            # M2: attnT_pair = t16^T @ qkT_pair in one 128-deep matmul.
            attnT = bigp.tile([P, KT, T], F16, tag="big", name="attnT")
            EH = JW // 2  # 192: even-dim half of a block

            def emit_qk(i, m, jw, qkR):
                ps = ps384.tile([P, JW], F32, tag="ps384")
                for k in range(KT):
                    nc.tensor.matmul(
                        ps[:], latT[:, k, ts(i, P)],
                        wT["qk_w"][:, k, ts(jw, JW)],
                        start=(k == 0), stop=(k == KT - 1))
                xb = qpp.tile([P, JW], F16, tag="qkp")
                nc.vector.tensor_add(xb[:], ps[:], bc_qkb[:, ts(jw, JW)])
                # rope: block layout [E_all(192) | O_all(192)]
                ms = [rmp.tile([P, EH], F16, tag="ropem", name=f"rm{n}")
                      for n in range(4)]
                for mt, (half, tn) in zip(ms, [(0, "cpE"), (1, "spE"),
                                               (0, "spO"), (1, "cpO")]):
                    nc.vector.tensor_mul(
                        mt[:], xb[:, half * EH:(half + 1) * EH],
                        tabs[tn][:, m, :])
                # un-permute to per-head [rot1(32) | rot2(32)]
                ov = qkR[:, m, ts(jw, JW)].rearrange(
                    "p (h c e) -> p c h e", c=2, e=HD // 2)
                mv = [x[:].rearrange("p (h e) -> p h e", e=HD // 2)
                      for x in ms]
                nc.vector.tensor_sub(ov[:, 0], mv[0], mv[1])
                nc.gpsimd.tensor_add(ov[:, 1], mv[2], mv[3])

            def emit_v(i, m, jw, vtm):
                ps = ps384.tile([P, JW], F32, tag="ps384")
                for k in range(KT):
                    nc.tensor.matmul(
                        ps[:], latT[:, k, ts(i, P)],
                        wT["v_w"][:, k, ts(jw, JW)],
                        start=(k == 0), stop=False)
                nc.tensor.matmul(
                    ps[:], ones1[:], brow["v_b"][:, ts(jw, JW)],
                    start=False, stop=True)
                nc.scalar.activation(vtm[:, m, ts(jw, JW)], ps[:], AF.Copy)

            for b in range(B_LOC):
                qkR = bigh.tile([P, TPB, D], F16, tag="bigh",
                                name=f"qkR{b}")
                qkT = bigh.tile([P, KT, SEQ], F16, tag="bigh",
                                name=f"qkT{b}")
                vtm = bigh.tile([P, TPB, D], F16, tag="bigh",
                                name=f"v{b}")
                for m in range(TPB):
                    i = b * TPB + m
                    with nc.named_scope("qk"):
                        for jw in range(NJW):
                            emit_qk(i, m, jw, qkR)
                    with nc.named_scope("v"):
                        for jw in range(NJW):
                            emit_v(i, m, jw, vtm)
                    with nc.named_scope("qkt"):
                        nc.scalar.dma_start(qkT[:, :, ts(m, P)],
                                            qkR[:, m, :], transpose=True)
                for hp in range(KT):
                    with nc.named_scope("attn_m1"):
                        pt = psA.tile([P, P], F32, tag="psA")
                        for m in range(TPB):
                            nc.tensor.matmul(
                                pt[:], qkR[:, m, ts(hp, P)],
                                vtm[:, m, ts(hp, P)],
                                start=(m == 0), stop=(m == TPB - 1))
                        t16 = t16p.tile([P, P], F16, tag="t16")
                        nc.vector.tensor_mul(t16[:], pt[:], mask[:])
                    with nc.named_scope("attn_m2"):
                        for nw in range(2):
                            ps = ps512.tile([P, TW], F32, tag="ps512")
                            nc.tensor.matmul(
                                ps[:], t16[:], qkT[:, hp, ds(nw * TW, TW)],
                                start=True, stop=True)
                            nc.scalar.activation(
                                attnT[:, hp, ds(b * SEQ + nw * TW, TW)],
                                ps[:], AF.Copy)

            # ------------- gate + output projection + residual -------
            out_dst = out_t.ap().rearrange(
                "b (w c p) d -> (b w) p c d", p=P, c=2)
            xr_src = x_in.ap().rearrange(
                "b (w c p) d -> (b w) p c d", p=P, c=2)
            with nc.named_scope("out"):
                for i in range(NT):
                    if i % 2 == 0:
                        xr4 = xrp.tile([P, 2, D], F32, tag="xres")
                        nc.sync.dma_start(xr4[:], xr_src[i // 2])
                    xr = xr4[:, i % 2]
                    for jw in range(NJW):
                        psg = ps384.tile([P, JW], F32, tag="ps384")
                        for k in range(KT):
                            nc.tensor.matmul(
                                psg[:], xnT[:, k, ts(i, P)],
                                wT["gate_w"][:, k, ts(jw, JW)],
                                start=(k == 0), stop=False)
                        nc.tensor.matmul(
                            psg[:], ones1[:], brow["gate_b"][:, ts(jw, JW)],
                            start=False, stop=True)
                        g16 = g16p.tile([P, JW], F16, tag="g16")
                        nc.scalar.activation(g16[:], psg[:], AF.Sigmoid)

                        ps = ps384.tile([P, JW], F32, tag="ps384")
                        for k in range(KT):
                            nc.tensor.matmul(
                                ps[:], attnT[:, k, ts(i, P)],
                                wT["out_w"][:, k, ts(jw, JW)],
                                start=(k == 0), stop=False)
                        nc.tensor.matmul(
                            ps[:], ones1[:], brow["out_b"][:, ts(jw, JW)],
                            start=False, stop=True)
                        ao = aop.tile([P, JW], F16, tag="ao")
                        nc.vector.tensor_mul(ao[:], ps[:], g16[:])
                        nc.gpsimd.tensor_add(xr[:, ds(jw * JW, JW)], ao[:],
                                             xr[:, ds(jw * JW, JW)])
                    if i % 2 == 1:
                        nc.scalar.dma_start(out_dst[i // 2], xr4[:])

    nc.finalize()
    return nc


_NC = None


def _get_nc():
    global _NC
    if _NC is None:
        _NC = build_nc()
    return _NC


def _qk_perm():
    """Per 384-block: all even head-dims (6 heads x 32), then all odds."""
    perm = []
    for jb in range(NJW):
        base = jb * JW
        for par in (0, 1):
            for h in range(JW // HD):
                perm.extend(base + h * HD + np.arange(par, HD, 2))
    return np.asarray(perm)


def make_in_maps(inputs, n_cores=8):
    x = np.ascontiguousarray(inputs["x"], dtype=np.float32)
    f32 = lambda a: np.asarray(a, dtype=np.float32)
    ln_w, ln_b = f32(inputs["ln_w"]), f32(inputs["ln_b"])
    shared = {"rope_emb": np.ascontiguousarray(f32(inputs["rope_emb"]))}

    # fold the LN affine into the two consumers of x_norm (host-side prep)
    enc_w = f32(inputs["enc_w"]) * ln_w[None, :]
    gate_w = f32(inputs["gate_w"]) * ln_w[None, :]
    shared["enc_b"] = np.ascontiguousarray(
        f32(inputs["enc_b"]) + f32(inputs["enc_w"]) @ ln_b)
    shared["gate_b"] = np.ascontiguousarray(
        f32(inputs["gate_b"]) + f32(inputs["gate_w"]) @ ln_b)
    shared["v_b"] = np.ascontiguousarray(f32(inputs["v_b"]))
    shared["out_b"] = np.ascontiguousarray(f32(inputs["out_b"]))

    # qk: block-wise [evens | odds] output-feature permutation (layout prep
    # for contiguous on-device rope slices)
    perm = _qk_perm()
    qk_w = f32(inputs["qk_w"])[perm]
    shared["qk_b"] = np.ascontiguousarray(f32(inputs["qk_b"])[perm])

    ws = {"enc_w": enc_w, "qk_w": qk_w, "v_w": f32(inputs["v_w"]),
          "out_w": f32(inputs["out_w"]), "gate_w": gate_w}
    for nm in W_NAMES:
        # device consumes W^T ([d, j]) in fp16; transpose/cast are
        # host-side layout/precision prep (device math is fp16 anyway)
        shared[nm] = np.ascontiguousarray(ws[nm].T.astype(np.float16))

    in_maps = []
    for c in range(n_cores):
        m = dict(shared)
        m["x"] = np.ascontiguousarray(x[c * B_LOC:(c + 1) * B_LOC])
        in_maps.append(m)
    return in_maps


def kernel(**inputs):
    nc = _get_nc()
    n_cores = 8
    in_maps = make_in_maps(inputs, n_cores)
    res = bass_utils.run_bass_kernel_spmd(
        nc, in_maps, core_ids=list(range(n_cores)))
    return np.concatenate([r["out"] for r in res.results], axis=0)


# revision 51
# speedup vs baseline: 1.0008x; 1.0008x over previous
"""Trainium2 Bass kernel for nn_BDHBlock (pre-LN latent block with
softmax-free attention and sigmoid gating).

Sharding: data-parallel over batch B=16 across 8 cores (2 per core).
No collectives; outputs are concatenated on the host.

Per-core math (B_loc=2, N=1024, D=768, H=12, HD=64), all matmuls fp16
with fp32 PSUM accumulation:
  xn   = LayerNorm(x)                      (ln_w/ln_b folded into
                                            enc/gate weights host-side)
  lat  = relu(xn @ enc_w'.T + enc_b')      (feature-major)
  qk   = rope(lat @ qk_w.T + qk_b) / sqrt(sqrt(HD))   (token-major)
  v    = lat @ v_w.T + v_b                 (token-major)
  T_h  = qk_h^T @ v_h     per (b, head-pair), fused 128-wide matmuls
  attn_h = qk_h @ T_h     (== (qk qk^T/8) v by associativity)
  out  = x + sigmoid(xn @ gate_w'.T + gate_b') * (attn @ out_w.T + out_b)

Layout notes:
- All SBUF<->SBUF transposes (xn -> xnT, qk -> qkT) go through the DMA
  XBAR transpose (fp16), not the PE array.
- Weights are loaded with casting gpsimd DMAs (f32 DRAM -> f16 SBUF).
- qk output features are host-permuted per 384-block to [all-even dims,
  all-odd dims] so the rope multiplies are contiguous 2D ops; the
  gpsimd add/sub writes un-permute back to per-head [rot1|rot2] order.
"""

import os
import sys

for _p in ("/opt/trn_rl_repo", "/root/.axon_site/_ro/trn_rl_repo"):
    if os.path.isdir(_p) and _p not in sys.path:
        sys.path.insert(0, _p)

import math
import numpy as np

import concourse.bass as bass
import concourse.mybir as mybir
from concourse import bacc
from concourse import bass_utils
from concourse.bass import ts, ds
from concourse.tile import TileContext

F32 = mybir.dt.float32
F16 = mybir.dt.float16
AF = mybir.ActivationFunctionType

P = 128          # partitions
D = 768
KT = D // P      # 6 d-tiles
B_LOC = 2        # batch elements per core
SEQ = 1024
T = B_LOC * SEQ  # 2048 tokens per core
NT = T // P      # 16 token tiles
TPB = SEQ // P   # 8 token tiles per batch element
TW = 512         # token window (feature-major matmul free dim)
NTW = T // TW    # 4
JW = 384         # feature window (token-major matmul free dim)
NJW = D // JW    # 2
H = 12
HD = 64
EPS = 1e-5
QK_SCALE = 1.0 / math.sqrt(math.sqrt(HD))  # applied twice => 1/sqrt(HD)

W_NAMES = ["enc_w", "qk_w", "v_w", "out_w", "gate_w"]


def _trig_coefs():
    """Power-series coefficients for sin(x)=x*S(x^2), cos(x)=C(x^2) on
    |x|<=6 (rope_emb is N(0,1); the ACT Sin LUT is unusable there)."""
    xs = np.linspace(1e-8, 6.0, 40001)
    u = xs ** 2
    cheb = np.polynomial.chebyshev
    for deg in range(7, 14):
        s = cheb.cheb2poly(cheb.chebfit(u, np.sin(xs) / xs, deg))
        c = cheb.cheb2poly(cheb.chebfit(u, np.cos(xs), deg))
        es = np.abs(np.polyval(s[::-1], u) * xs - np.sin(xs)).max()
        ec = np.abs(np.polyval(c[::-1], u) - np.cos(xs)).max()
        if max(es, ec) < 4e-5:
            break
    return [float(v) for v in s], [float(v) for v in c]


SIN_COEF, COS_COEF = _trig_coefs()


def build_nc():
    nc = bacc.Bacc("TRN2", target_bir_lowering=False, debug=False)

    x_in = nc.dram_tensor("x", [B_LOC, SEQ, D], F32, kind="ExternalInput")
    rope_in = nc.dram_tensor("rope_emb", [SEQ, HD], F32, kind="ExternalInput")
    vecs = {}
    for nm in ["enc_b", "qk_b", "v_b", "out_b", "gate_b"]:
        vecs[nm] = nc.dram_tensor(nm, [D], F32, kind="ExternalInput")
    # weights arrive host-cast to fp16 (and host-transposed): halves DMA
    # bytes and removes on-device casts
    w_in = {nm: nc.dram_tensor(nm, [D, D], F16, kind="ExternalInput")
            for nm in W_NAMES}
    out_t = nc.dram_tensor("out", [B_LOC, SEQ, D], F32, kind="ExternalOutput")

    x_flat = x_in.ap().rearrange("b n d -> (b n) d")
    out_flat = out_t.ap().rearrange("b n d -> (b n) d")

    with TileContext(nc) as tc:
        with (
            tc.tile_pool(name="consts", bufs=1) as cp,
            tc.tile_pool(name="wrot", bufs=3) as wrot,
            tc.tile_pool(name="big", bufs=2) as bigp,
            tc.tile_pool(name="bigh", bufs=3) as bigh,
            tc.tile_pool(name="xin", bufs=2) as xip,
            tc.tile_pool(name="xres", bufs=2) as xrp,
            tc.tile_pool(name="xn16", bufs=2) as xnp,
            tc.tile_pool(name="stats", bufs=2) as stp,
            tc.tile_pool(name="qkp", bufs=2) as qpp,
            tc.tile_pool(name="ropem", bufs=4) as rmp,
            tc.tile_pool(name="t16", bufs=2) as t16p,
            tc.tile_pool(name="g16", bufs=2) as g16p,
            tc.tile_pool(name="ao", bufs=2) as aop,
            tc.tile_pool(name="ps512", bufs=3, space="PSUM") as ps512,
            tc.tile_pool(name="ps384", bufs=3, space="PSUM") as ps384,
            tc.tile_pool(name="psA", bufs=2, space="PSUM") as psA,
        ):
            # ---------------- constants / weight loads ----------------
            x_src = x_in.ap().rearrange(
                "b (w c p) d -> (b w) p c d", p=P, c=4)
            with nc.named_scope("prep"):
                # LN consumes x in fp16 (stats/affine precision is ample):
                # casting gpsimd DMA halves DVE stats cost and keeps the
                # sync queue free.  Window 0 split per tile: LN tile 0 is
                # the whole kernel's prologue.
                xt4 = xip.tile([P, 4, D], F16, tag="xin", name="xw0")
                for c in range(4):
                    nc.gpsimd.dma_start(xt4[:, c], x_src[0][:, c])
                eps_t = cp.tile([P, 1], F32, tag="epsc")
                nc.vector.memset(eps_t[:], EPS)
                # block-diagonal head mask for the fused 2-head M1 drain
                mask = cp.tile([P, P], F16, tag="hmask")
                nc.vector.memset(mask[:], 0.0)
                nc.vector.memset(mask[0:HD, 0:HD], 1.0)
                nc.vector.memset(mask[HD:P, HD:P], 1.0)
                ones1 = cp.tile([1, P], F16, tag="ones1")
                nc.vector.memset(ones1[:], 1.0)

                # enc bias, per-partition layout [128, KT]
                encb = cp.tile([P, KT], F32, tag="encb")
                nc.sync.dma_start(
                    encb[:], vecs["enc_b"].ap().rearrange("(k p) -> p k", p=P))
                rp = cp.tile([P, TPB, HD], F32, tag="ropein")
                nc.sync.dma_start(
                    rp[:], rope_in.ap().rearrange("(t p) d -> p t d", p=P))
                # qk bias as a broadcast tile: folded into the drain add
                bc_qkb = cp.tile([P, D], F16, tag="bc_qkb")
                nc.gpsimd.dma_start(
                    out=bc_qkb[:],
                    in_=vecs["qk_b"].ap()[None, :].to_broadcast((P, D)))

                # weights: casting DMA (f32 DRAM -> f16 SBUF), whole weight
                # in one descriptor batch; host has pre-transposed to W^T.
                # ring of 3 slots: enc->out, qk->gate, v (alloc order)
                wT = {}
                for nm in W_NAMES:
                    wT[nm] = wrot.tile([P, KT, D], F16, tag="wT",
                                       name=f"wT_{nm}")
                w_src = {nm: w_in[nm].ap().rearrange("(k p) j -> p k j", p=P)
                         for nm in W_NAMES}
                # enc_w per k-tile on sync: enc w0's k-loop can start as
                # soon as the k=0 slice lands
                for k in range(KT):
                    nc.sync.dma_start(wT["enc_w"][:, k, :],
                                      w_src["enc_w"][:, k, :])
                for nm in ["qk_w", "v_w"]:
                    nc.gpsimd.dma_start(wT[nm][:], w_src[nm])

                # bias rows for the K=1 ones-matmul trick
                brow = {}
                for nm in ["v_b", "gate_b", "out_b"]:
                    brow[nm] = cp.tile([1, D], F16, tag=f"brow_{nm}",
                                       name=f"brow_{nm}")
                    nc.gpsimd.dma_start(brow[nm][:], vecs[nm].ap()[None, :])

            # ------------- rope tables (gpsimd, off the LN path) -------
            # tabs: 4 tables [P, TPB, 6*32] fp16, head-replicated so the
            # rope multiplies are plain 2D [128, 192] ops.
            #   cpE=cos(even)*s  spE=sin(even)*s  spO=sin(odd)*s  cpO=cos(odd)*s
            NH = JW // HD  # 6 heads per block
            tabs = {}
            for tn in ["cpE", "spE", "spO", "cpO"]:
                tabs[tn] = cp.tile([P, TPB, NH * (HD // 2)], F16, tag=tn,
                                   name=f"tab_{tn}")
            # Trig tables: f32 Horner on DVE (gpsimd is ~8x slower at
            # tensor_scalar; fp16 Horner loses ~0.5 abs err to cancellation
            # at large u).  Emitted as closures interleaved into the back
            # half of the LN loop so the serial chain hides in DVE slack.
            u = cp.tile([P, TPB, HD], F32, tag="ropeu")
            sin_a = cp.tile([P, TPB, HD], F32, tag="ropesin")
            cos_a = cp.tile([P, TPB, HD], F32, tag="ropecos")
            trig_steps = []

            def _trig_build():
                yield lambda: nc.vector.tensor_mul(u[:], rp[:], rp[:])

                def horner_steps(coef, out, fin):
                    # recurrence on m_k = (m_{k+1} + c_{k+1}) * u via
                    # scalar_tensor_tensor: one DVE op per step
                    yield lambda: nc.vector.tensor_scalar_mul(
                        out[:], u[:], coef[-1])
                    for cf in coef[-2:0:-1]:
                        yield lambda cf=cf: nc.vector.scalar_tensor_tensor(
                            out[:], out[:], cf, u[:],
                            op0=mybir.AluOpType.add,
                            op1=mybir.AluOpType.mult)
                    if fin is None:  # cos: m_0 + c_0
                        yield lambda: nc.vector.tensor_scalar_add(
                            out[:], out[:], coef[0])
                    else:  # sin: (m_0 + c_0) * x
                        yield lambda: nc.vector.scalar_tensor_tensor(
                            out[:], out[:], coef[0], fin,
                            op0=mybir.AluOpType.add,
                            op1=mybir.AluOpType.mult)

                yield from horner_steps(SIN_COEF, sin_a, rp[:])
                yield from horner_steps(COS_COEF, cos_a, None)
                srcs = {"cpE": (cos_a, 0), "spE": (sin_a, 0),
                        "spO": (sin_a, 1), "cpO": (cos_a, 1)}
                for tn, (src, par) in srcs.items():
                    def mk(tn=tn, src=src, par=par):
                        dstv = tabs[tn][:].rearrange(
                            "p t (h e) -> p t h e", e=HD // 2)
                        srcv = src[:, :, None, par::2].to_broadcast(
                            (P, TPB, NH, HD // 2))
                        nc.vector.tensor_scalar_mul(dstv, srcv, QK_SCALE)
                    yield mk
            trig_steps = list(_trig_build())

            # xn^T: feature-major [128, KT, T]; lives until the gate GEMMs.
            xnT = cp.tile([P, KT, T], F16, tag="xnT")

            # ---------------- LayerNorm + encoder (interleaved) --------
            # x loads batched 4 tiles per DMA; the full [128,768] -> xnT
            # transpose is one 48-tile XBAR DMA (HWDGE overhead ~630ns/DMA
            # dwarfs the 14ns/tile transfer, so batch aggressively).
            latT = bigp.tile([P, KT, T], F16, tag="big", name="latT")
            xt4s = {0: xt4}
            for i in range(NT):
                with nc.named_scope("ln"):
                    if i % 4 == 0 and i // 4 + 1 < 4:
                        nxt = xip.tile([P, 4, D], F16, tag="xin",
                                       name=f"xw{i // 4 + 1}")
                        nc.gpsimd.dma_start(nxt[:], x_src[i // 4 + 1])
                        xt4s[i // 4 + 1] = nxt
                    xt = xt4s[i // 4][:, i % 4]
                    xg = xt.rearrange("p (s c) -> p s c", c=384)
                    stats = stp.tile([P, 2, 6], F32, tag="bnstats")
                    for s in range(2):
                        nc.vector.bn_stats(stats[:, s, :], xg[:, s, :])
                    mv = stp.tile([P, 2], F32, tag="bnmv")
                    nc.vector.bn_aggr(mv[:], stats[:])
                    rs = stp.tile([P, 1], F32, tag="rstd")
                    nc.scalar.activation(rs[:], mv[:, 1:2], AF.Sqrt,
                                         bias=eps_t[:])
                    nc.vector.reciprocal(rs[:], rs[:])
                    nb = stp.tile([P, 1], F32, tag="negmurs")
                    nc.vector.tensor_scalar(
                        nb[:], mv[:, 0:1], rs[:], -1.0,
                        op0=mybir.AluOpType.mult, op1=mybir.AluOpType.mult)
                    xn16 = xnp.tile([P, D], F16, tag="xn16")
                    nc.vector.tensor_scalar(
                        xn16[:], xt, rs[:], nb[:],
                        op0=mybir.AluOpType.mult, op1=mybir.AluOpType.add)
                    # all transposes on one queue (scalar): the XBAR is a
                    # single shared resource; concurrent transposes from
                    # two queues interleave tiles and corrupt output.
                    # scalar also keeps them off the x-load (sync) queue.
                    nc.scalar.dma_start(xnT[:, :, ts(i, P)], xn16[:],
                                        transpose=True)
                if i >= 4:  # trig chain rides LN's DVE slack
                    with nc.named_scope("trig"):
                        for fn in trig_steps[(i - 4) * 2:(i - 3) * 2]:
                            fn()
                if i % 4 == 3:
                    tw = i // 4
                    with nc.named_scope("enc"):
                        for j in range(KT):
                            ps = ps512.tile([P, TW], F32, tag="ps512")
                            for k in range(KT):
                                nc.tensor.matmul(
                                    ps[:], wT["enc_w"][:, k, ts(j, P)],
                                    xnT[:, k, ts(tw, TW)],
                                    start=(k == 0), stop=(k == KT - 1))
                            nc.scalar.activation(latT[:, j, ts(tw, TW)],
                                                 ps[:], AF.Relu,
                                                 bias=encb[:, j:j + 1])

            with nc.named_scope("trig"):
                for fn in trig_steps[(NT - 4) * 2:]:
                    fn()
            # hoist the sigmoid ACT-table swap off the out-phase critical
            # path: last Sqrt user was LN tile 15, first real Sigmoid is in
            # the out phase; swap now while ACT has slack.
            warm_sg = cp.tile([P, 1], F32, tag="warmsg")
            nc.scalar.activation(warm_sg[:], eps_t[:], AF.Sigmoid)

            # late weight loads (slots freed by enc/qk phases)
            nc.gpsimd.dma_start(wT["out_w"][:], w_src["out_w"])
            nc.gpsimd.dma_start(wT["gate_w"][:], w_src["gate_w"])

            # ------------ qk (rope) + v + attention, per batch ---------
            # Per-batch [P, TPB, D]-sized buffers (half of T) ring-reuse
            # between the two batch elements to fit SBUF.
            # M1: fused head-pair [128,128] = qk_pair^T @ v_pair; diag
            # 64-blocks are T_hA, T_hB; off-diag garbage masked at drain.
            # M2: attnT_pair = t16^T @ qkT_pair in one 128-deep matmul.
            attnT = bigp.tile([P, KT, T], F16, tag="big", name="attnT")
            EH = JW // 2  # 192: even-dim half of a block

            def emit_qk(i, m, jw, qkR):
                ps = ps384.tile([P, JW], F32, tag="ps384")
                for k in range(KT):
                    nc.tensor.matmul(
                        ps[:], latT[:, k, ts(i, P)],
                        wT["qk_w"][:, k, ts(jw, JW)],
                        start=(k == 0), stop=(k == KT - 1))
                xb = qpp.tile([P, JW], F16, tag="qkp")
                nc.vector.tensor_add(xb[:], ps[:], bc_qkb[:, ts(jw, JW)])
                # rope: block layout [E_all(192) | O_all(192)]
                ms = [rmp.tile([P, EH], F16, tag="ropem", name=f"rm{n}")
                      for n in range(4)]
                for mt, (half, tn) in zip(ms, [(0, "cpE"), (1, "spE"),
                                               (0, "spO"), (1, "cpO")]):
                    nc.vector.tensor_mul(
                        mt[:], xb[:, half * EH:(half + 1) * EH],
                        tabs[tn][:, m, :])
                # un-permute to per-head [rot1(32) | rot2(32)]
                ov = qkR[:, m, ts(jw, JW)].rearrange(
                    "p (h c e) -> p c h e", c=2, e=HD // 2)
                mv = [x[:].rearrange("p (h e) -> p h e", e=HD // 2)
                      for x in ms]
                nc.vector.tensor_sub(ov[:, 0], mv[0], mv[1])
                nc.gpsimd.tensor_add(ov[:, 1], mv[2], mv[3])

            def emit_v(i, m, jw, vtm):
                ps = ps384.tile([P, JW], F32, tag="ps384")
                for k in range(KT):
                    nc.tensor.matmul(
                        ps[:], latT[:, k, ts(i, P)],
                        wT["v_w"][:, k, ts(jw, JW)],
                        start=(k == 0), stop=False)
                nc.tensor.matmul(
                    ps[:], ones1[:], brow["v_b"][:, ts(jw, JW)],
                    start=False, stop=True)
                nc.scalar.activation(vtm[:, m, ts(jw, JW)], ps[:], AF.Copy)

            for b in range(B_LOC):
                qkR = bigh.tile([P, TPB, D], F16, tag="bigh",
                                name=f"qkR{b}")
                qkT = bigh.tile([P, KT, SEQ], F16, tag="bigh",
                                name=f"qkT{b}")
                vtm = bigh.tile([P, TPB, D], F16, tag="bigh",
                                name=f"v{b}")
                for m in range(TPB):
                    i = b * TPB + m
                    with nc.named_scope("qk"):
                        for jw in range(NJW):
                            emit_qk(i, m, jw, qkR)
                    with nc.named_scope("v"):
                        for jw in range(NJW):
                            emit_v(i, m, jw, vtm)
                    with nc.named_scope("qkt"):
                        nc.scalar.dma_start(qkT[:, :, ts(m, P)],
                                            qkR[:, m, :], transpose=True)
                for hp in range(KT):
                    with nc.named_scope("attn_m1"):
                        pt = psA.tile([P, P], F32, tag="psA")
                        for m in range(TPB):
                            nc.tensor.matmul(
                                pt[:], qkR[:, m, ts(hp, P)],
                                vtm[:, m, ts(hp, P)],
                                start=(m == 0), stop=(m == TPB - 1))
                        t16 = t16p.tile([P, P], F16, tag="t16")
                        nc.vector.tensor_mul(t16[:], pt[:], mask[:])
                    with nc.named_scope("attn_m2"):
                        for nw in range(2):
                            ps = ps512.tile([P, TW], F32, tag="ps512")
                            nc.tensor.matmul(
                                ps[:], t16[:], qkT[:, hp, ds(nw * TW, TW)],
                                start=True, stop=True)
                            nc.scalar.activation(
                                attnT[:, hp, ds(b * SEQ + nw * TW, TW)],
                                ps[:], AF.Copy)

            # ------------- gate + output projection + residual -------
            out_dst = out_t.ap().rearrange(
                "b (w c p) d -> (b w) p c d", p=P, c=2)
            xr_src = x_in.ap().rearrange(
                "b (w c p) d -> (b w) p c d", p=P, c=2)
            with nc.named_scope("out"):
                for i in range(NT):
                    if i % 2 == 0:
                        xr4 = xrp.tile([P, 2, D], F32, tag="xres")
                        nc.sync.dma_start(xr4[:], xr_src[i // 2])
                    xr = xr4[:, i % 2]
                    for jw in range(NJW):
                        psg = ps384.tile([P, JW], F32, tag="ps384")
                        for k in range(KT):
                            nc.tensor.matmul(
                                psg[:], xnT[:, k, ts(i, P)],
                                wT["gate_w"][:, k, ts(jw, JW)],
                                start=(k == 0), stop=False)
                        nc.tensor.matmul(
                            psg[:], ones1[:], brow["gate_b"][:, ts(jw, JW)],
                            start=False, stop=True)
                        g16 = g16p.tile([P, JW], F16, tag="g16")
                        nc.scalar.activation(g16[:], psg[:], AF.Sigmoid)

                        ps = ps384.tile([P, JW], F32, tag="ps384")
                        for k in range(KT):
                            nc.tensor.matmul(
                                ps[:], attnT[:, k, ts(i, P)],
                                wT["out_w"][:, k, ts(jw, JW)],
                                start=(k == 0), stop=False)
                        nc.tensor.matmul(
                            ps[:], ones1[:], brow["out_b"][:, ts(jw, JW)],
                            start=False, stop=True)
                        ao = aop.tile([P, JW], F16, tag="ao")
                        nc.vector.tensor_mul(ao[:], ps[:], g16[:])
                        nc.gpsimd.tensor_add(xr[:, ds(jw * JW, JW)], ao[:],
                                             xr[:, ds(jw * JW, JW)])
                    if i % 2 == 1:
                        nc.scalar.dma_start(out_dst[i // 2], xr4[:])

    nc.finalize()
    return nc


_NC = None


def _get_nc():
    global _NC
    if _NC is None:
        _NC = build_nc()
    return _NC


def _qk_perm():
    """Per 384-block: all even head-dims (6 heads x 32), then all odds."""
    perm = []
    for jb in range(NJW):
        base = jb * JW
        for par in (0, 1):
            for h in range(JW // HD):
                perm.extend(base + h * HD + np.arange(par, HD, 2))
    return np.asarray(perm)


def make_in_maps(inputs, n_cores=8):
    x = np.ascontiguousarray(inputs["x"], dtype=np.float32)
    f32 = lambda a: np.asarray(a, dtype=np.float32)
    ln_w, ln_b = f32(inputs["ln_w"]), f32(inputs["ln_b"])
    shared = {"rope_emb": np.ascontiguousarray(f32(inputs["rope_emb"]))}

    # fold the LN affine into the two consumers of x_norm (host-side prep)
    enc_w = f32(inputs["enc_w"]) * ln_w[None, :]
    gate_w = f32(inputs["gate_w"]) * ln_w[None, :]
    shared["enc_b"] = np.ascontiguousarray(
        f32(inputs["enc_b"]) + f32(inputs["enc_w"]) @ ln_b)
    shared["gate_b"] = np.ascontiguousarray(
        f32(inputs["gate_b"]) + f32(inputs["gate_w"]) @ ln_b)
    shared["v_b"] = np.ascontiguousarray(f32(inputs["v_b"]))
    shared["out_b"] = np.ascontiguousarray(f32(inputs["out_b"]))

    # qk: block-wise [evens | odds] output-feature permutation (layout prep
    # for contiguous on-device rope slices)
    perm = _qk_perm()
    qk_w = f32(inputs["qk_w"])[perm]
    shared["qk_b"] = np.ascontiguousarray(f32(inputs["qk_b"])[perm])

    ws = {"enc_w": enc_w, "qk_w": qk_w, "v_w": f32(inputs["v_w"]),
          "out_w": f32(inputs["out_w"]), "gate_w": gate_w}
    for nm in W_NAMES:
        # device consumes W^T ([d, j]) in fp16; transpose/cast are
        # host-side layout/precision prep (device math is fp16 anyway)
        shared[nm] = np.ascontiguousarray(ws[nm].T.astype(np.float16))

    in_maps = []
    for c in range(n_cores):
        m = dict(shared)
        m["x"] = np.ascontiguousarray(x[c * B_LOC:(c + 1) * B_LOC])
        in_maps.append(m)
    return in_maps


def kernel(**inputs):
    nc = _get_nc()
    n_cores = 8
    in_maps = make_in_maps(inputs, n_cores)
    res = bass_utils.run_bass_kernel_spmd(
        nc, in_maps, core_ids=list(range(n_cores)))
    return np.concatenate([r["out"] for r in res.results], axis=0)


# revision 57
# speedup vs baseline: 1.1395x; 1.1386x over previous
"""Trainium2 Bass kernel for nn_BDHBlock (pre-LN latent block with
softmax-free attention and sigmoid gating).

Sharding: data-parallel over batch B=16 across 8 cores (2 per core).
No collectives; outputs are concatenated on the host.

Per-core math (B_loc=2, N=1024, D=768, H=12, HD=64), all matmuls fp16
with fp32 PSUM accumulation:
  xn   = LayerNorm(x)                      (ln_w/ln_b folded into
                                            enc/gate weights host-side)
  lat  = relu(xn @ enc_w'.T + enc_b')      (feature-major)
  qk   = rope(lat @ qk_w.T + qk_b) / sqrt(sqrt(HD))   (token-major)
  v    = lat @ v_w.T + v_b                 (token-major)
  T_h  = qk_h^T @ v_h     per (b, head-pair), fused 128-wide matmuls
  attn_h = qk_h @ T_h     (== (qk qk^T/8) v by associativity)
  out  = x + sigmoid(xn @ gate_w'.T + gate_b') * (attn @ out_w.T + out_b)

Layout notes:
- All SBUF<->SBUF transposes (xn -> xnT, qk -> qkT) go through the DMA
  XBAR transpose (fp16), not the PE array.
- Weights are loaded with casting gpsimd DMAs (f32 DRAM -> f16 SBUF).
- qk output features are host-permuted per 384-block to [all-even dims,
  all-odd dims] so the rope multiplies are contiguous 2D ops; the
  gpsimd add/sub writes un-permute back to per-head [rot1|rot2] order.
"""

import os
import sys

for _p in ("/opt/trn_rl_repo", "/root/.axon_site/_ro/trn_rl_repo"):
    if os.path.isdir(_p) and _p not in sys.path:
        sys.path.insert(0, _p)

import math
import numpy as np

import concourse.bass as bass
import concourse.mybir as mybir
from concourse import bacc
from concourse import bass_utils
from concourse.bass import ts, ds
from concourse.tile import TileContext

F32 = mybir.dt.float32
F16 = mybir.dt.float16
AF = mybir.ActivationFunctionType

P = 128          # partitions
D = 768
KT = D // P      # 6 d-tiles
B_LOC = 2        # batch elements per core
SEQ = 1024
T = B_LOC * SEQ  # 2048 tokens per core
NT = T // P      # 16 token tiles
TPB = SEQ // P   # 8 token tiles per batch element
TW = 512         # token window (feature-major matmul free dim)
NTW = T // TW    # 4
JW = 384         # feature window (token-major matmul free dim)
NJW = D // JW    # 2
H = 12
HD = 64
EPS = 1e-5
QK_SCALE = 1.0 / math.sqrt(math.sqrt(HD))  # applied twice => 1/sqrt(HD)

W_NAMES = ["enc_w", "qk_w", "v_w", "out_w", "gate_w"]


def _trig_coefs():
    """Power-series coefficients for sin(x)=x*S(x^2), cos(x)=C(x^2) on
    |x|<=6 (rope_emb is N(0,1); the ACT Sin LUT is unusable there)."""
    xs = np.linspace(1e-8, 6.0, 40001)
    u = xs ** 2
    cheb = np.polynomial.chebyshev
    for deg in range(7, 14):
        s = cheb.cheb2poly(cheb.chebfit(u, np.sin(xs) / xs, deg))
        c = cheb.cheb2poly(cheb.chebfit(u, np.cos(xs), deg))
        es = np.abs(np.polyval(s[::-1], u) * xs - np.sin(xs)).max()
        ec = np.abs(np.polyval(c[::-1], u) - np.cos(xs)).max()
        if max(es, ec) < 4e-5:
            break
    return [float(v) for v in s], [float(v) for v in c]


SIN_COEF, COS_COEF = _trig_coefs()


def build_nc():
    nc = bacc.Bacc("TRN2", target_bir_lowering=False, debug=False)

    x_in = nc.dram_tensor("x", [B_LOC, SEQ, D], F32, kind="ExternalInput")
    rope_in = nc.dram_tensor("rope_emb", [SEQ, HD], F32, kind="ExternalInput")
    vecs = {}
    for nm in ["enc_b", "qk_b", "v_b", "out_b", "gate_b"]:
        vecs[nm] = nc.dram_tensor(nm, [D], F32, kind="ExternalInput")
    # weights arrive host-cast to fp16 (and host-transposed): halves DMA
    # bytes and removes on-device casts
    w_in = {nm: nc.dram_tensor(nm, [D, D], F16, kind="ExternalInput")
            for nm in W_NAMES}
    out_t = nc.dram_tensor("out", [B_LOC, SEQ, D], F32, kind="ExternalOutput")

    x_flat = x_in.ap().rearrange("b n d -> (b n) d")
    out_flat = out_t.ap().rearrange("b n d -> (b n) d")

    with TileContext(nc) as tc:
        with (
            tc.tile_pool(name="consts", bufs=1) as cp,
            tc.tile_pool(name="wrot", bufs=3) as wrot,
            tc.tile_pool(name="big", bufs=2) as bigp,
            tc.tile_pool(name="bigh", bufs=3) as bigh,
            tc.tile_pool(name="xin", bufs=2) as xip,
            tc.tile_pool(name="xres", bufs=2) as xrp,
            tc.tile_pool(name="xn16", bufs=2) as xnp,
            tc.tile_pool(name="stats", bufs=2) as stp,
            tc.tile_pool(name="qkp", bufs=2) as qpp,
            tc.tile_pool(name="ropem", bufs=4) as rmp,
            tc.tile_pool(name="t16", bufs=2) as t16p,
            tc.tile_pool(name="g16", bufs=2) as g16p,
            tc.tile_pool(name="ao", bufs=2) as aop,
            tc.tile_pool(name="ps512", bufs=3, space="PSUM") as ps512,
            tc.tile_pool(name="ps384", bufs=3, space="PSUM") as ps384,
            tc.tile_pool(name="psA", bufs=2, space="PSUM") as psA,
        ):
            # ---------------- constants / weight loads ----------------
            x_src = x_in.ap().rearrange(
                "b (w c p) d -> (b w) p c d", p=P, c=4)
            with nc.named_scope("prep"):
                # LN consumes x in fp16 (stats/affine precision is ample):
                # casting gpsimd DMA halves DVE stats cost and keeps the
                # sync queue free.  Window 0 split per tile: LN tile 0 is
                # the whole kernel's prologue.
                xt4 = xip.tile([P, 4, D], F16, tag="xin", name="xw0")
                for c in range(4):
                    nc.gpsimd.dma_start(xt4[:, c], x_src[0][:, c])
                xw1 = xip.tile([P, 4, D], F16, tag="xin", name="xw1")
                nc.gpsimd.dma_start(xw1[:], x_src[1])
                eps_t = cp.tile([P, 1], F32, tag="epsc")
                nc.vector.memset(eps_t[:], EPS)
                # block-diagonal head mask for the fused 2-head M1 drain
                mask = cp.tile([P, P], F16, tag="hmask")
                nc.vector.memset(mask[:], 0.0)
                nc.vector.memset(mask[0:HD, 0:HD], 1.0)
                nc.vector.memset(mask[HD:P, HD:P], 1.0)
                ones1 = cp.tile([1, P], F16, tag="ones1")
                nc.vector.memset(ones1[:], 1.0)

                # enc bias, per-partition layout [128, KT]
                encb = cp.tile([P, KT], F32, tag="encb")
                nc.sync.dma_start(
                    encb[:], vecs["enc_b"].ap().rearrange("(k p) -> p k", p=P))
                rp = cp.tile([P, TPB, HD], F32, tag="ropein")
                nc.sync.dma_start(
                    rp[:], rope_in.ap().rearrange("(t p) d -> p t d", p=P))
                # qk bias as a broadcast tile: folded into the drain add
                bc_qkb = cp.tile([P, D], F16, tag="bc_qkb")
                nc.gpsimd.dma_start(
                    out=bc_qkb[:],
                    in_=vecs["qk_b"].ap()[None, :].to_broadcast((P, D)))

                # weights: casting DMA (f32 DRAM -> f16 SBUF), whole weight
                # in one descriptor batch; host has pre-transposed to W^T.
                # ring of 3 slots: enc->out, qk->gate, v (alloc order)
                wT = {}
                for nm in W_NAMES:
                    wT[nm] = wrot.tile([P, KT, D], F16, tag="wT",
                                       name=f"wT_{nm}")
                w_src = {nm: w_in[nm].ap().rearrange("(k p) j -> p k j", p=P)
                         for nm in W_NAMES}
                # enc_w per k-tile on sync: enc w0's k-loop can start as
                # soon as the k=0 slice lands
                for k in range(KT):
                    nc.sync.dma_start(wT["enc_w"][:, k, :],
                                      w_src["enc_w"][:, k, :])
                for nm in ["qk_w", "v_w"]:
                    nc.gpsimd.dma_start(wT[nm][:], w_src[nm])

                # bias rows for the K=1 ones-matmul trick
                brow = {}
                for nm in ["v_b", "gate_b", "out_b"]:
                    brow[nm] = cp.tile([1, D], F16, tag=f"brow_{nm}",
                                       name=f"brow_{nm}")
                    nc.gpsimd.dma_start(brow[nm][:], vecs[nm].ap()[None, :])

            # ------------- rope tables (gpsimd, off the LN path) -------
            # tabs: 4 tables [P, TPB, 6*32] fp16, head-replicated so the
            # rope multiplies are plain 2D [128, 192] ops.
            #   cpE=cos(even)*s  spE=sin(even)*s  spO=sin(odd)*s  cpO=cos(odd)*s
            NH = JW // HD  # 6 heads per block
            tabs = {}
            for tn in ["cpE", "spE", "spO", "cpO"]:
                tabs[tn] = cp.tile([P, TPB, NH * (HD // 2)], F16, tag=tn,
                                   name=f"tab_{tn}")
            # Trig tables: f32 Horner on DVE (gpsimd is ~8x slower at
            # tensor_scalar; fp16 Horner loses ~0.5 abs err to cancellation
            # at large u).  Emitted as closures interleaved into the back
            # half of the LN loop so the serial chain hides in DVE slack.
            u = cp.tile([P, TPB, HD], F32, tag="ropeu")
            sin_a = cp.tile([P, TPB, HD], F32, tag="ropesin")
            cos_a = cp.tile([P, TPB, HD], F32, tag="ropecos")
            trig_steps = []

            def _trig_build():
                yield lambda: nc.vector.tensor_mul(u[:], rp[:], rp[:])

                def horner_steps(coef, out, fin):
                    # recurrence on m_k = (m_{k+1} + c_{k+1}) * u via
                    # scalar_tensor_tensor: one DVE op per step
                    yield lambda: nc.vector.tensor_scalar_mul(
                        out[:], u[:], coef[-1])
                    for cf in coef[-2:0:-1]:
                        yield lambda cf=cf: nc.vector.scalar_tensor_tensor(
                            out[:], out[:], cf, u[:],
                            op0=mybir.AluOpType.add,
                            op1=mybir.AluOpType.mult)
                    if fin is None:  # cos: m_0 + c_0
                        yield lambda: nc.vector.tensor_scalar_add(
                            out[:], out[:], coef[0])
                    else:  # sin: (m_0 + c_0) * x
                        yield lambda: nc.vector.scalar_tensor_tensor(
                            out[:], out[:], coef[0], fin,
                            op0=mybir.AluOpType.add,
                            op1=mybir.AluOpType.mult)

                yield from horner_steps(SIN_COEF, sin_a, rp[:])
                yield from horner_steps(COS_COEF, cos_a, None)
                srcs = {"cpE": (cos_a, 0), "spE": (sin_a, 0),
                        "spO": (sin_a, 1), "cpO": (cos_a, 1)}
                for tn, (src, par) in srcs.items():
                    def mk(tn=tn, src=src, par=par):
                        dstv = tabs[tn][:].rearrange(
                            "p t (h e) -> p t h e", e=HD // 2)
                        srcv = src[:, :, None, par::2].to_broadcast(
                            (P, TPB, NH, HD // 2))
                        nc.vector.tensor_scalar_mul(dstv, srcv, QK_SCALE)
                    yield mk
            trig_steps = list(_trig_build())

            # xn^T: feature-major [128, KT, T]; lives until the gate GEMMs.
            xnT = cp.tile([P, KT, T], F16, tag="xnT")

            # ---------------- LayerNorm + encoder (interleaved) --------
            # x loads batched 4 tiles per DMA; the full [128,768] -> xnT
            # transpose is one 48-tile XBAR DMA (HWDGE overhead ~630ns/DMA
            # dwarfs the 14ns/tile transfer, so batch aggressively).
            latT = bigp.tile([P, KT, T], F16, tag="big", name="latT")
            xt4s = {0: xt4, 1: xw1}
            for i in range(NT):
                with nc.named_scope("ln"):
                    if i % 4 == 0 and i // 4 + 2 < 4:
                        nxt = xip.tile([P, 4, D], F16, tag="xin",
                                       name=f"xw{i // 4 + 2}")
                        nc.gpsimd.dma_start(nxt[:], x_src[i // 4 + 2])
                        xt4s[i // 4 + 2] = nxt
                    xt = xt4s[i // 4][:, i % 4]
                    xg = xt.rearrange("p (s c) -> p s c", c=384)
                    stats = stp.tile([P, 2, 6], F32, tag="bnstats")
                    for s in range(2):
                        nc.vector.bn_stats(stats[:, s, :], xg[:, s, :])
                    mv = stp.tile([P, 2], F32, tag="bnmv")
                    nc.vector.bn_aggr(mv[:], stats[:])
                    rs = stp.tile([P, 1], F32, tag="rstd")
                    nc.scalar.activation(rs[:], mv[:, 1:2], AF.Sqrt,
                                         bias=eps_t[:])
                    nc.vector.reciprocal(rs[:], rs[:])
                    nb = stp.tile([P, 1], F32, tag="negmurs")
                    nc.vector.tensor_scalar(
                        nb[:], mv[:, 0:1], rs[:], -1.0,
                        op0=mybir.AluOpType.mult, op1=mybir.AluOpType.mult)
                    xn16 = xnp.tile([P, D], F16, tag="xn16")
                    nc.vector.tensor_scalar(
                        xn16[:], xt, rs[:], nb[:],
                        op0=mybir.AluOpType.mult, op1=mybir.AluOpType.add)
            # all transposes on one queue (sync): the XBAR is a single
            # shared resource; concurrent transposes from two queues
            # interleave tiles and corrupt output.  They must NOT share a
            # queue with other time-critical work: a DMA instruction holds
            # the issuing engine's sequencer while waiting for its input.
                    nc.sync.dma_start(xnT[:, :, ts(i, P)], xn16[:],
                                      transpose=True)
                if i >= 4:  # trig chain rides LN's DVE slack
                    with nc.named_scope("trig"):
                        for fn in trig_steps[(i - 4) * 2:(i - 3) * 2]:
                            fn()
                if i % 4 == 3:
                    tw = i // 4
                    with nc.named_scope("enc"):
                        for j in range(KT):
                            ps = ps512.tile([P, TW], F32, tag="ps512")
                            for k in range(KT):
                                nc.tensor.matmul(
                                    ps[:], wT["enc_w"][:, k, ts(j, P)],
                                    xnT[:, k, ts(tw, TW)],
                                    start=(k == 0), stop=(k == KT - 1))
                            nc.scalar.activation(latT[:, j, ts(tw, TW)],
                                                 ps[:], AF.Relu,
                                                 bias=encb[:, j:j + 1])

            with nc.named_scope("trig"):
                for fn in trig_steps[(NT - 4) * 2:]:
                    fn()
            # hoist the sigmoid ACT-table swap off the out-phase critical
            # path: last Sqrt user was LN tile 15, first real Sigmoid is in
            # the out phase; swap now while ACT has slack.
            warm_sg = cp.tile([P, 1], F32, tag="warmsg")
            nc.scalar.activation(warm_sg[:], eps_t[:], AF.Sigmoid)

            # late weight loads (slots freed by enc/qk phases)
            nc.gpsimd.dma_start(wT["out_w"][:], w_src["out_w"])
            nc.gpsimd.dma_start(wT["gate_w"][:], w_src["gate_w"])

            # ------------ qk (rope) + v + attention, per batch ---------
            # Per-batch [P, TPB, D]-sized buffers (half of T) ring-reuse
            # between the two batch elements to fit SBUF.
            # M1: fused head-pair [128,128] = qk_pair^T @ v_pair; diag
            # 64-blocks are T_hA, T_hB; off-diag garbage masked at drain.
            # M2: attnT_pair = t16^T @ qkT_pair in one 128-deep matmul.
            attnT = bigp.tile([P, KT, T], F16, tag="big", name="attnT")
            EH = JW // 2  # 192: even-dim half of a block

            def emit_qk(i, m, jw, qkR):
                ps = ps384.tile([P, JW], F32, tag="ps384")
                for k in range(KT):
                    nc.tensor.matmul(
                        ps[:], latT[:, k, ts(i, P)],
                        wT["qk_w"][:, k, ts(jw, JW)],
                        start=(k == 0), stop=(k == KT - 1))
                xb = qpp.tile([P, JW], F16, tag="qkp")
                nc.vector.tensor_add(xb[:], ps[:], bc_qkb[:, ts(jw, JW)])
                # rope: block layout [E_all(192) | O_all(192)]
                ms = [rmp.tile([P, EH], F16, tag="ropem", name=f"rm{n}")
                      for n in range(4)]
                for mt, (half, tn) in zip(ms, [(0, "cpE"), (1, "spE"),
                                               (0, "spO"), (1, "cpO")]):
                    nc.vector.tensor_mul(
                        mt[:], xb[:, half * EH:(half + 1) * EH],
                        tabs[tn][:, m, :])
                # un-permute to per-head [rot1(32) | rot2(32)]
                ov = qkR[:, m, ts(jw, JW)].rearrange(
                    "p (h c e) -> p c h e", c=2, e=HD // 2)
                mv = [x[:].rearrange("p (h e) -> p h e", e=HD // 2)
                      for x in ms]
                nc.vector.tensor_sub(ov[:, 0], mv[0], mv[1])
                nc.gpsimd.tensor_add(ov[:, 1], mv[2], mv[3])

            def emit_v(i, m, jw, vtm):
                ps = ps384.tile([P, JW], F32, tag="ps384")
                for k in range(KT):
                    nc.tensor.matmul(
                        ps[:], latT[:, k, ts(i, P)],
                        wT["v_w"][:, k, ts(jw, JW)],
                        start=(k == 0), stop=False)
                nc.tensor.matmul(
                    ps[:], ones1[:], brow["v_b"][:, ts(jw, JW)],
                    start=False, stop=True)
                nc.scalar.activation(vtm[:, m, ts(jw, JW)], ps[:], AF.Copy)

            for b in range(B_LOC):
                qkR = bigh.tile([P, TPB, D], F16, tag="bigh",
                                name=f"qkR{b}")
                qkT = bigh.tile([P, KT, SEQ], F16, tag="bigh",
                                name=f"qkT{b}")
                vtm = bigh.tile([P, TPB, D], F16, tag="bigh",
                                name=f"v{b}")
                for m in range(TPB):
                    i = b * TPB + m
                    with nc.named_scope("qk"):
                        for jw in range(NJW):
                            emit_qk(i, m, jw, qkR)
                    with nc.named_scope("v"):
                        for jw in range(NJW):
                            emit_v(i, m, jw, vtm)
                    with nc.named_scope("qkt"):
                        nc.sync.dma_start(qkT[:, :, ts(m, P)],
                                          qkR[:, m, :], transpose=True)
                for hp in range(KT):
                    with nc.named_scope("attn_m1"):
                        pt = psA.tile([P, P], F32, tag="psA")
                        for m in range(TPB):
                            nc.tensor.matmul(
                                pt[:], qkR[:, m, ts(hp, P)],
                                vtm[:, m, ts(hp, P)],
                                start=(m == 0), stop=(m == TPB - 1))
                        t16 = t16p.tile([P, P], F16, tag="t16")
                        nc.vector.tensor_mul(t16[:], pt[:], mask[:])
                    with nc.named_scope("attn_m2"):
                        for nw in range(2):
                            ps = ps512.tile([P, TW], F32, tag="ps512")
                            nc.tensor.matmul(
                                ps[:], t16[:], qkT[:, hp, ds(nw * TW, TW)],
                                start=True, stop=True)
                            nc.scalar.activation(
                                attnT[:, hp, ds(b * SEQ + nw * TW, TW)],
                                ps[:], AF.Copy)

            # ------------- gate + output projection + residual -------
            out_dst = out_t.ap().rearrange(
                "b (w c p) d -> (b w) p c d", p=P, c=2)
            xr_src = x_in.ap().rearrange(
                "b (w c p) d -> (b w) p c d", p=P, c=2)
            with nc.named_scope("out"):
                for i in range(NT):
                    if i % 2 == 0:
                        xr4 = xrp.tile([P, 2, D], F32, tag="xres")
                        nc.gpsimd.dma_start(xr4[:], xr_src[i // 2])
                    xr = xr4[:, i % 2]
                    for jw in range(NJW):
                        psg = ps384.tile([P, JW], F32, tag="ps384")
                        for k in range(KT):
                            nc.tensor.matmul(
                                psg[:], xnT[:, k, ts(i, P)],
                                wT["gate_w"][:, k, ts(jw, JW)],
                                start=(k == 0), stop=False)
                        nc.tensor.matmul(
                            psg[:], ones1[:], brow["gate_b"][:, ts(jw, JW)],
                            start=False, stop=True)
                        g16 = g16p.tile([P, JW], F16, tag="g16")
                        nc.scalar.activation(g16[:], psg[:], AF.Sigmoid)

                        ps = ps384.tile([P, JW], F32, tag="ps384")
                        for k in range(KT):
                            nc.tensor.matmul(
                                ps[:], attnT[:, k, ts(i, P)],
                                wT["out_w"][:, k, ts(jw, JW)],
                                start=(k == 0), stop=False)
                        nc.tensor.matmul(
                            ps[:], ones1[:], brow["out_b"][:, ts(jw, JW)],
                            start=False, stop=True)
                        ao = aop.tile([P, JW], F16, tag="ao")
                        nc.vector.tensor_mul(ao[:], ps[:], g16[:])
                        nc.gpsimd.tensor_add(xr[:, ds(jw * JW, JW)], ao[:],
                                             xr[:, ds(jw * JW, JW)])
                    if i % 2 == 1:
                        # out-writes ride the gpsimd queue right behind the
                        # residual adds: same-queue order makes the data
                        # dependency free (no sequencer stall)
                        nc.gpsimd.dma_start(out_dst[i // 2], xr4[:])

    nc.finalize()
    return nc


_NC = None


def _get_nc():
    global _NC
    if _NC is None:
        _NC = build_nc()
    return _NC


def _qk_perm():
    """Per 384-block: all even head-dims (6 heads x 32), then all odds."""
    perm = []
    for jb in range(NJW):
        base = jb * JW
        for par in (0, 1):
            for h in range(JW // HD):
                perm.extend(base + h * HD + np.arange(par, HD, 2))
    return np.asarray(perm)


def make_in_maps(inputs, n_cores=8):
    x = np.ascontiguousarray(inputs["x"], dtype=np.float32)
    f32 = lambda a: np.asarray(a, dtype=np.float32)
    ln_w, ln_b = f32(inputs["ln_w"]), f32(inputs["ln_b"])
    shared = {"rope_emb": np.ascontiguousarray(f32(inputs["rope_emb"]))}

    # fold the LN affine into the two consumers of x_norm (host-side prep)
    enc_w = f32(inputs["enc_w"]) * ln_w[None, :]
    gate_w = f32(inputs["gate_w"]) * ln_w[None, :]
    shared["enc_b"] = np.ascontiguousarray(
        f32(inputs["enc_b"]) + f32(inputs["enc_w"]) @ ln_b)
    shared["gate_b"] = np.ascontiguousarray(
        f32(inputs["gate_b"]) + f32(inputs["gate_w"]) @ ln_b)
    shared["v_b"] = np.ascontiguousarray(f32(inputs["v_b"]))
    shared["out_b"] = np.ascontiguousarray(f32(inputs["out_b"]))

    # qk: block-wise [evens | odds] output-feature permutation (layout prep
    # for contiguous on-device rope slices)
    perm = _qk_perm()
    qk_w = f32(inputs["qk_w"])[perm]
    shared["qk_b"] = np.ascontiguousarray(f32(inputs["qk_b"])[perm])

    ws = {"enc_w": enc_w, "qk_w": qk_w, "v_w": f32(inputs["v_w"]),
          "out_w": f32(inputs["out_w"]), "gate_w": gate_w}
    for nm in W_NAMES:
        # device consumes W^T ([d, j]) in fp16; transpose/cast are
        # host-side layout/precision prep (device math is fp16 anyway)
        shared[nm] = np.ascontiguousarray(ws[nm].T.astype(np.float16))

    in_maps = []
    for c in range(n_cores):
        m = dict(shared)
        m["x"] = np.ascontiguousarray(x[c * B_LOC:(c + 1) * B_LOC])
        in_maps.append(m)
    return in_maps


def kernel(**inputs):
    nc = _get_nc()
    n_cores = 8
    in_maps = make_in_maps(inputs, n_cores)
    res = bass_utils.run_bass_kernel_spmd(
        nc, in_maps, core_ids=list(range(n_cores)))
    return np.concatenate([r["out"] for r in res.results], axis=0)
